# revision 32
# baseline (speedup 1.0000x reference)
"""Causal self-attention (RoPE) Trainium2 Bass kernel, SPMD over 8 NeuronCores.

Sharding: core i -> batch b = i // 4, head group hg = i % 4 (4 heads each).
Per core: QKV projections (f32r matmuls), RoPE via rotated-weight projections,
scores computed transposed [k, q] (softmax denominator via ones-column in V),
causal block skipping, out-proj partial. Host sums the 4 partials per batch.
"""
import numpy as np
import ml_dtypes

import concourse.mybir as mybir
import concourse.tile as tile
from concourse import bacc
from concourse.bass_utils import run_bass_kernel_spmd

B, S, D = 2, 2048, 1024
H, HD = 16, 64
NCORES = 8
GROUPS = NCORES // B          # 4 tensor-parallel cores per batch
HLOC = H // GROUPS            # 4 heads per core
FLOC = HLOC * HD              # 256 local features
P = 128
SC = 512                      # s-chunk (q-chunk) width
NCH = S // SC                 # 4 chunks
KBLK = S // P                 # 16 key blocks
DKB = D // P                  # 8 contraction blocks for projections
NEG = -200.0                  # clamped mask value; exp(-200+s) == 0 in fp32

F32 = mybir.dt.float32
F32R = mybir.dt.float32r
BF16 = mybir.dt.bfloat16
AF = mybir.ActivationFunctionType

# stage dtypes (bf16 streams 2 cols/cycle on PE + fast weight load; f32r is
# ~11-bit-mantissa fp32 at 1 col/cycle)
PROJ_BF16 = False    # xT/wqk/wro/wv/wo/aT (projection + out-proj matmuls)
ATT_BF16 = False     # qT/kT/v_aug/pT/ident/mtri/mask (attention matmuls)

_CACHE: dict = {}
_LAST_RESULTS = None
_LAST_IN_MAPS = None


def _build(causal: bool, has_mask: bool, has_bias: bool, repeat: int = 1):
    nc = bacc.Bacc("TRN2", target_bir_lowering=False, debug=False,
                   num_devices=NCORES)
    dp = nc.declare_dram_parameter
    pdt = BF16 if PROJ_BF16 else F32R
    adt = BF16 if ATT_BF16 else F32R
    io = {
        "xT":   dp("xT",   [D, S],        pdt, isOutput=False),
        "wqk":  dp("wqk",  [D, 2 * FLOC], pdt, isOutput=False),
        "wro":  dp("wro",  [D, 2 * FLOC], pdt, isOutput=False),
        "wv":   dp("wv",   [D, FLOC],     pdt, isOutput=False),
        "wo":   dp("wo",   [FLOC, D],     pdt, isOutput=False),
        "cos2": dp("cos2", [P, S],        F32,  isOutput=False),
        "sin2": dp("sin2", [P, S],        F32,  isOutput=False),
        "ident": dp("ident", [P, P],      adt, isOutput=False),
        "y":    dp("y",    [S, D],        F32,  isOutput=True),
    }
    if causal:
        io["mtri"] = dp("mtri", [P, P], adt, isOutput=False)
    elif has_mask:
        io["maskT"] = dp("maskT", [S, S], adt, isOutput=False)
    if has_bias:
        io["bqk"] = dp("bqk", [1, 2 * FLOC], F32R, isOutput=False)
        io["bro"] = dp("bro", [1, 2 * FLOC], F32R, isOutput=False)
        io["bv"] = dp("bv", [1, FLOC], F32R, isOutput=False)

    with tile.TileContext(nc) as tc, \
         nc.allow_low_precision(reason="float32r rounding for PE operands"):
        _emit(nc, tc, io, causal, has_mask, has_bias, repeat)
    nc.finalize()
    return nc


def _emit(nc, tc, io, causal, has_mask, has_bias, repeat=1):
    pdt = BF16 if PROJ_BF16 else F32R
    adt = BF16 if ATT_BF16 else F32R
    from contextlib import ExitStack
    ctx = ExitStack()
    with ctx:
        wpool = ctx.enter_context(tc.tile_pool(name="weights", bufs=1))
        xpool = ctx.enter_context(tc.tile_pool(name="xt", bufs=2))
        qkpool = ctx.enter_context(tc.tile_pool(name="qk", bufs=1))
        vpool = ctx.enter_context(tc.tile_pool(name="v", bufs=1))
        apool = ctx.enter_context(tc.tile_pool(name="aT", bufs=1))
        tmppool = ctx.enter_context(tc.tile_pool(name="tmp", bufs=2))
        ptpool = ctx.enter_context(tc.tile_pool(name="pT", bufs=3))
        smpool = ctx.enter_context(tc.tile_pool(name="small", bufs=2))
        ypool = ctx.enter_context(tc.tile_pool(name="y", bufs=2))
        mmps = ctx.enter_context(tc.tile_pool(name="mmps", bufs=3, space="PSUM"))
        scps = ctx.enter_context(tc.tile_pool(name="scps", bufs=2, space="PSUM"))
        pvps = ctx.enter_context(tc.tile_pool(name="pvps", bufs=1, space="PSUM"))
        if (not causal) and has_mask:
            mkpool = ctx.enter_context(tc.tile_pool(name="mask", bufs=1))

        # ---- constant / weight loads (ordered by first use) --------------
        wqk_t, wro_t, wv_t = [], [], []
        for kb in range(DKB):
            t = wpool.tile([P, 2 * FLOC], pdt, tag=f"wqk{kb}")
            nc.sync.dma_start(t[:], io["wqk"][kb * P:(kb + 1) * P, :])
            wqk_t.append(t)
        for kb in range(DKB):
            t = wpool.tile([P, 2 * FLOC], pdt, tag=f"wro{kb}")
            nc.sync.dma_start(t[:], io["wro"][kb * P:(kb + 1) * P, :])
            wro_t.append(t)
        cos_t, sin_t = [], []
        for c in range(NCH):
            t = wpool.tile([P, SC], F32, tag=f"cos{c}")
            nc.sync.dma_start(t[:], io["cos2"][:, c * SC:(c + 1) * SC])
            cos_t.append(t)
            t = wpool.tile([P, SC], F32, tag=f"sin{c}")
            nc.sync.dma_start(t[:], io["sin2"][:, c * SC:(c + 1) * SC])
            sin_t.append(t)
        for kb in range(DKB):
            t = wpool.tile([P, FLOC], pdt, tag=f"wv{kb}")
            nc.sync.dma_start(t[:], io["wv"][kb * P:(kb + 1) * P, :])
            wv_t.append(t)
        ident = wpool.tile([P, P], adt, tag="ident")
        nc.sync.dma_start(ident[:], io["ident"][:])
        if causal:
            mtri = wpool.tile([P, P], adt, tag="mtri")
            nc.sync.dma_start(mtri[:], io["mtri"][:])
        wo_t = []
        for fb in range(FLOC // P):
            t = wpool.tile([P, D], pdt, tag=f"wo{fb}")
            nc.sync.dma_start(t[:], io["wo"][fb * P:(fb + 1) * P, :])
            wo_t.append(t)
        if has_bias:
            bqk_t = wpool.tile([1, 2 * FLOC], F32R, tag="bqk")
            nc.sync.dma_start(bqk_t[:], io["bqk"][:])
            bro_t = wpool.tile([1, 2 * FLOC], F32R, tag="bro")
            nc.sync.dma_start(bro_t[:], io["bro"][:])
            bv_t = wpool.tile([1, FLOC], F32R, tag="bv")
            nc.sync.dma_start(bv_t[:], io["bv"][:])
        ones_f = wpool.tile([1, SC], F32, tag="onesf")
        nc.vector.memset(ones_f[:], 1.0)
        ones_r = wpool.tile([1, SC], F32R, tag="onesr")
        nc.vector.tensor_copy(ones_r[:], ones_f[:])
        onecol_f = wpool.tile([P, 1], F32, tag="onecol")
        nc.vector.memset(onecol_f[:], 1.0)

        # ---- per-chunk emission: QKV(c) -> attention(c) -> outproj(c) ----
        # qk tiles: qT_{m}_{c}, kT_{m}_{c}  [128, SC] f32r (m: head pair)
        qk_tiles = {}
        v_tiles = {}
        a_tiles = {}

        def emit_qkv(c):
            xt = []
            for kb in range(DKB):
                t = xpool.tile([P, SC], pdt, tag=f"xt{kb}")
                nc.sync.dma_start(
                    t[:], io["xT"][kb * P:(kb + 1) * P, c * SC:(c + 1) * SC])
                xt.append(t)
            # Q (m=0,1) and K (m=2,3) m-tiles, plus rotated versions
            for m in range(4):
                ps_a = mmps.tile([P, SC], F32, tag="mm")
                ps_b = mmps.tile([P, SC], F32, tag="mm")
                for kb in range(DKB):
                    nc.tensor.matmul(
                        ps_a[:], wqk_t[kb][:, m * P:(m + 1) * P], xt[kb][:],
                        start=(kb == 0), stop=(kb == DKB - 1 and not has_bias))
                if has_bias:
                    nc.tensor.matmul(ps_a[:], bqk_t[0:1, m * P:(m + 1) * P],
                                     ones_r[0:1, :], start=False, stop=True)
                for kb in range(DKB):
                    nc.tensor.matmul(
                        ps_b[:], wro_t[kb][:, m * P:(m + 1) * P], xt[kb][:],
                        start=(kb == 0), stop=(kb == DKB - 1 and not has_bias))
                if has_bias:
                    nc.tensor.matmul(ps_b[:], bro_t[0:1, m * P:(m + 1) * P],
                                     ones_r[0:1, :], start=False, stop=True)
                kind = "qT" if m < 2 else "kT"
                dest = qkpool.tile([P, SC], adt, tag=f"{kind}{m % 2}_{c}")
                qk_tiles[(kind, m % 2, c)] = dest
                t1 = tmppool.tile([P, SC], F32, tag="ropea")
                t2 = tmppool.tile([P, SC], F32, tag="ropeb")
                nc.vector.tensor_mul(t1[:], ps_a[:], cos_t[c][:])
                nc.vector.tensor_mul(t2[:], ps_b[:], sin_t[c][:])
                nc.vector.tensor_add(dest[:], t1[:], t2[:])
            # V for the 4 s-tiles of this chunk, augmented with ones column
            for st in range(SC // P):
                s_t = c * (SC // P) + st
                ps = mmps.tile([P, SC], F32, tag="mm")
                for kb in range(DKB):
                    nc.tensor.matmul(
                        ps[:, :FLOC], xt[kb][:, st * P:(st + 1) * P],
                        wv_t[kb][:],
                        start=(kb == 0), stop=(kb == DKB - 1 and not has_bias))
                if has_bias:
                    nc.tensor.matmul(ps[:, :FLOC], ones_r[0:1, :P],
                                     bv_t[0:1, :], start=False, stop=True)
                vt = vpool.tile([P, HLOC * (HD + 1)], adt, tag=f"v{s_t}")
                v_tiles[s_t] = vt
                vview = vt[:].rearrange("p (h w) -> p h w", w=HD + 1)
                nc.vector.tensor_copy(
                    vview[:, :, :HD],
                    ps[:, :FLOC].rearrange("p (h w) -> p h w", w=HD))
                nc.vector.tensor_copy(
                    vview[:, :, HD],
                    onecol_f[:, 0:1].to_broadcast((P, HLOC)))

        def emit_attn(c):
            for hp in range(HLOC // 2):
                at = apool.tile([P, SC], pdt, tag=f"aT{hp}_{c}")
                a_tiles[(hp, c)] = at
            for h in range(HLOC):
                hp, hh = h // 2, (h % 2) * HD
                kbs = list(range(4 * c + 4)) if causal else list(range(KBLK))
                pv = pvps.tile([P, SC], F32, tag="pv")
                for j0 in range(0, len(kbs), 2):
                    pair = kbs[j0:j0 + 2]
                    sc_ps = scps.tile([P, 2 * SC], F32, tag="sc")
                    spans = []
                    for j, kb in enumerate(pair):
                        diag = causal and (kb // 4 == c)
                        qq0 = (kb % 4) * P if diag else 0
                        base = j * SC
                        kt = qk_tiles[("kT", hp, kb // 4)]
                        qt = qk_tiles[("qT", hp, c)]
                        nc.tensor.matmul(
                            sc_ps[:, base + qq0:base + SC],
                            kt[hh:hh + HD, (kb % 4) * P:(kb % 4 + 1) * P],
                            qt[hh:hh + HD, qq0:SC],
                            start=True, stop=not (diag or ((not causal) and has_mask)))
                        if diag:
                            nc.tensor.matmul(
                                sc_ps[:, base + qq0:base + qq0 + P],
                                ident[:], mtri[:], start=False, stop=True)
                        elif (not causal) and has_mask:
                            mk = mkpool.tile([P, SC], adt, tag=f"mk{kb}")
                            if h == 0:
                                nc.sync.dma_start(
                                    mk[:], io["maskT"][kb * P:(kb + 1) * P,
                                                       c * SC:(c + 1) * SC])
                            nc.tensor.matmul(sc_ps[:, base:base + SC],
                                             ident[:], mk[:],
                                             start=False, stop=True)
                        spans.append((kb, qq0, base))
                    pt = ptpool.tile([P, 2 * SC], adt, tag="pt")
                    if all(q == 0 for _, q, _ in spans) and len(spans) == 2:
                        nc.scalar.activation(pt[:], sc_ps[:], AF.Exp)
                    else:
                        for kb, qq0, base in spans:
                            nc.scalar.activation(
                                pt[:, base + qq0:base + SC],
                                sc_ps[:, base + qq0:base + SC], AF.Exp)
                    for kb, qq0, base in spans:
                        nc.tensor.matmul(
                            pv[0:HD + 1, qq0:SC],
                            v_tiles[kb][:, h * (HD + 1):(h + 1) * (HD + 1)],
                            pt[:, base + qq0:base + SC],
                            start=(kb == 0), stop=(kb == kbs[-1]))
                recip = smpool.tile([1, SC], F32R, tag="recip")
                nc.vector.reciprocal(recip[:], pv[HD:HD + 1, :])
                bc_ps = mmps.tile([P, SC], F32, tag="mm")
                nc.tensor.matmul(bc_ps[0:HD, :], ones_r[0:1, :HD],
                                 recip[0:1, :], start=True, stop=True)
                bc = smpool.tile([HD, SC], F32, tag="bc")
                nc.scalar.activation(bc[:], bc_ps[0:HD, :], AF.Copy)
                nc.vector.tensor_mul(a_tiles[(hp, c)][hh:hh + HD, :],
                                     pv[0:HD, :], bc[:])

        def emit_outproj(c):
            for st in range(SC // P):
                s_t = c * (SC // P) + st
                ysb = ypool.tile([P, D], F32, tag="ysb")
                for e in range(D // SC):
                    yps = mmps.tile([P, SC], F32, tag="mm")
                    for fb in range(FLOC // P):
                        nc.tensor.matmul(
                            yps[:], a_tiles[(fb, c)][:, st * P:(st + 1) * P],
                            wo_t[fb][:, e * SC:(e + 1) * SC],
                            start=(fb == 0), stop=(fb == FLOC // P - 1))
                    if e % 2 == 0:
                        nc.scalar.activation(ysb[:, e * SC:(e + 1) * SC],
                                             yps[:], AF.Copy)
                    else:
                        nc.vector.tensor_copy(ysb[:, e * SC:(e + 1) * SC],
                                              yps[:])
                nc.sync.dma_start(io["y"][s_t * P:(s_t + 1) * P, :], ysb[:])

        def emit_all():
            for c in range(NCH):
                emit_qkv(c)
                emit_attn(c)
                emit_outproj(c)

        if repeat == 1:
            emit_all()
        else:
            with tc.For_i(0, repeat, 1):
                emit_all()


class _Runner:
    """Cached shard_map+jit executable for one built Bass program.

    Mirrors bass2jax.run_bass_via_pjrt's multi-core path, but reuses the
    traced/jitted function across calls (run_bass_via_pjrt rebuilds it each
    time, costing seconds of retrace per call) and skips output donation
    (this kernel writes every element of y).
    """

    def __init__(self, nc):
        import jax
        import numpy as _np
        from jax.sharding import Mesh, PartitionSpec
        from jax.experimental.shard_map import shard_map
        from concourse import bass2jax as b2j
        from concourse import mybir as mb

        b2j.install_neuronx_cc_hook()
        self.jax = jax
        part_name = (nc.partition_id_tensor.name
                     if nc.partition_id_tensor else None)
        in_names, out_names, out_avals, zero_outs = [], [], [], []
        for alloc in nc.m.functions[0].allocations:
            if not isinstance(alloc, mb.MemoryLocationSet):
                continue
            name = alloc.memorylocations[0].name
            if alloc.kind == "ExternalInput":
                if name != part_name:
                    in_names.append(name)
            elif alloc.kind == "ExternalOutput":
                out_names.append(name)
                out_avals.append(jax.core.ShapedArray(
                    tuple(alloc.tensor_shape), mb.dt.np(alloc.dtype)))
                zero_outs.append(_np.zeros(tuple(alloc.tensor_shape),
                                           mb.dt.np(alloc.dtype)))
        n_params = len(in_names)
        all_names = in_names + out_names
        if part_name is not None:
            all_names = all_names + [part_name]
        self.in_names, self.out_names = in_names, out_names
        self.out_avals = out_avals

        def _body(*args):
            operands = list(args)
            if part_name is not None:
                operands.append(b2j.partition_id_tensor())
            return tuple(b2j._bass_exec_p.bind(
                *operands,
                out_avals=tuple(out_avals),
                in_names=tuple(all_names),
                out_names=tuple(out_names),
                lowering_input_output_aliases=(),
                sim_require_finite=True,
                sim_require_nnan=True,
                nc=nc,
            ))

        self._body = _body
        devices = jax.devices()[:NCORES]
        mesh = Mesh(_np.asarray(devices), ("core",))
        nin = n_params + len(out_names)
        self.fn = jax.jit(shard_map(
            _body, mesh=mesh,
            in_specs=(PartitionSpec("core"),) * nin,
            out_specs=(PartitionSpec("core"),) * len(out_names),
            check_rep=False))
        self.zero_concat = [
            _np.zeros((NCORES * z.shape[0], *z.shape[1:]), z.dtype)
            for z in zero_outs]

    def concat_inputs(self, in_maps):
        import numpy as _np
        return [
            _np.concatenate([_np.asarray(in_maps[c][nm])
                             for c in range(NCORES)], axis=0)
            for nm in self.in_names]

    def run_device(self, dev_args):
        if not hasattr(self, "_zero_dev"):
            self._zero_dev = [self.jax.device_put(z) for z in self.zero_concat]
        out = self.fn(*dev_args, *self._zero_dev)
        self.jax.block_until_ready(out)
        return out

    def time_device(self, dev_args, iters=48, reps=3):
        """Median per-iteration device time: async-dispatch K executions
        (per-device stream serializes), block once; difference vs 1 call."""
        import time as _t
        jax = self.jax
        if not hasattr(self, "_zero_dev"):
            self._zero_dev = [self.jax.device_put(z) for z in self.zero_concat]
        jax.block_until_ready(self.fn(*dev_args, *self._zero_dev))  # warm

        def run_k(k):
            t0 = _t.perf_counter()
            outs = [self.fn(*dev_args, *self._zero_dev) for _ in range(k)]
            jax.block_until_ready(outs)
            return _t.perf_counter() - t0

        est = []
        for _ in range(reps):
            t1 = run_k(1)
            tN = run_k(iters)
            est.append((tN - t1) / (iters - 1))
        est.sort()
        return est[len(est) // 2], est

    def __call__(self, in_maps):
        import numpy as _np
        self._last_concat = self.concat_inputs(in_maps)
        out_arrs = self.fn(*self._last_concat, *self.zero_concat)
        return [
            {nm: _np.asarray(out_arrs[i]).reshape(
                NCORES, *self.out_avals[i].shape)[c]
             for i, nm in enumerate(self.out_names)}
            for c in range(NCORES)
        ]


_RUNNERS: dict = {}


def _get_runner(nc):
    if id(nc) not in _RUNNERS:
        _RUNNERS[id(nc)] = _Runner(nc)
    return _RUNNERS[id(nc)]


def _rope_tables():
    inv_freq = (1.0 / (10000.0 ** (np.arange(0, HD, 2, dtype=np.float32) / HD)))
    t = np.arange(S, dtype=np.float32)
    freqs = np.outer(t, inv_freq).astype(np.float32)      # (S, HD/2)
    emb = np.concatenate([freqs, freqs], axis=-1)          # (S, HD)
    return np.cos(emb).astype(np.float32), np.sin(emb).astype(np.float32)


def _rot_weights(w_loc):
    """rotate_half on the output-feature rows of a local weight slice."""
    r = w_loc.reshape(HLOC, HD, D)
    out = np.concatenate([-r[:, HD // 2:, :], r[:, :HD // 2, :]], axis=1)
    return out.reshape(FLOC, D)


def kernel(x, attn_mask, Wq, bq, Wk, bk, Wv, bv, Wo, bo):
    global _LAST_RESULTS, _LAST_IN_MAPS
    x = np.asarray(x, np.float32)
    attn_mask = np.asarray(attn_mask, np.float32)
    Wq, Wk, Wv, Wo = (np.asarray(w, np.float32) for w in (Wq, Wk, Wv, Wo))
    bq, bk, bv, bo = (np.asarray(b, np.float32) for b in (bq, bk, bv, bo))

    tri = np.tril(np.ones((S, S), dtype=bool))
    causal = bool(np.all(attn_mask[tri] == 0.0)
                  and np.all(attn_mask[~tri] <= -1e8))
    has_mask = bool(np.any(attn_mask != 0.0))
    has_bias = bool(np.any(bq) or np.any(bk) or np.any(bv))

    key = (causal, has_mask, has_bias, PROJ_BF16, ATT_BF16)
    if key not in _CACHE:
        _CACHE[key] = _build(causal, has_mask, has_bias)
    nc = _CACHE[key]
    pnp = ml_dtypes.bfloat16 if PROJ_BF16 else np.float32
    anp = ml_dtypes.bfloat16 if ATT_BF16 else np.float32

    cos, sin = _rope_tables()                 # (S, HD)
    cosT = np.ascontiguousarray(cos.T)        # (HD, S)
    sinT = np.ascontiguousarray(sin.T)
    cos2 = np.concatenate([cosT, cosT], axis=0)   # (128, S)
    sin2 = np.concatenate([sinT, sinT], axis=0)

    scale = 1.0 / np.sqrt(np.float32(HD))
    in_maps = []
    for cid in range(NCORES):
        b, hg = cid // GROUPS, cid % GROUPS
        fs = slice(hg * FLOC, (hg + 1) * FLOC)
        wq_loc = Wq[fs] * scale
        wk_loc = Wk[fs]
        m = {
            "xT": np.ascontiguousarray(x[b].T).astype(pnp),
            "wqk": np.ascontiguousarray(
                np.concatenate([wq_loc, wk_loc], axis=0).T).astype(pnp),
            "wro": np.ascontiguousarray(
                np.concatenate([_rot_weights(wq_loc), _rot_weights(wk_loc)],
                               axis=0).T).astype(pnp),
            "wv": np.ascontiguousarray(Wv[fs].T).astype(pnp),
            "wo": np.ascontiguousarray(Wo[:, fs].T).astype(pnp),
            "cos2": cos2,
            "sin2": sin2,
            "ident": np.eye(P, dtype=np.float32).astype(anp),
        }
        if causal:
            m["mtri"] = np.where(np.tril(np.ones((P, P), dtype=bool)),
                                 0.0, NEG).astype(np.float32).T.copy().astype(anp)
        elif has_mask:
            m["maskT"] = np.ascontiguousarray(
                np.maximum(attn_mask.T, NEG).astype(np.float32)).astype(anp)
        if has_bias:
            bq_loc = bq[fs] * scale
            bk_loc = bk[fs]
            m["bqk"] = np.concatenate([bq_loc, bk_loc])[None, :].copy()
            rr = lambda v: np.concatenate(
                [-v.reshape(HLOC, HD)[:, HD // 2:],
                 v.reshape(HLOC, HD)[:, :HD // 2]], axis=1).reshape(-1)
            m["bro"] = np.concatenate([rr(bq_loc), rr(bk_loc)])[None, :].copy()
            m["bv"] = bv[fs][None, :].copy()
        in_maps.append(m)

    _LAST_IN_MAPS = in_maps
    results = _get_runner(nc)(in_maps)
    _LAST_RESULTS = results

    out = np.zeros((B, S, D), dtype=np.float32)
    for cid in range(NCORES):
        out[cid // GROUPS] += results[cid]["y"]
    if np.any(bo):
        out += bo[None, None, :]
    return out
